# revision 11
# baseline (speedup 1.0000x reference)
"""Trainium2 Bass kernel for ChannelAttention-SNN (LIF -> GAP -> 1x1conv -> BN
-> 1x1conv -> BN).

Contract: kernel(**inputs) takes the FULL unsharded inputs (as produced by
setup_inputs) and returns the FULL output [T, B, C, 1] float32.

Strategy (hardcoded for T=4, B=16, C=512, N=1024, Cr=64, 8 cores):
  - Data-parallel over B: core m processes b in {2m, 2m+1}.
  - LIF scan is unrolled over T in "P-space": P_t = 2^t * v_pre_t, so
      P_t = P_{t-1} * m_{t-1} + 2^{t-1} * x_t,   spike_t <=> P_t >= 2^t,
    which folds the 1/tau decay into the (free) scale of the ScalarE cast
    fp32->bf16. Per timestep the VectorE does: mask = (P < theta) with a
    fused free-dim count (accum_out) that directly yields the GAP sums,
    a mask multiply, and an add.
  - The conv/BN tail is pipelined with streaming: as soon as timestep t
    completes, the per-core h1 partial rows [2, 64] for t are matmul'd
    and AllGather'd (one small collective per t), transposed, and fed to
    incremental bn_stats.  Only the t=3 gather plus a short serial BN
    chain remains after streaming ends.
  - Algebra: b1, beta1, b2 all cancel inside the BNs.  With
      h1c = (h1 - mean1) * gamma1 / std1     (centered BN1, no shift)
    the BN2-centered activations are exactly h2c = h1c @ w2.T, so
      out = h2c * gamma2 / sqrt(mean_rows(h2c^2) + eps) + beta2.
    Every core computes the tail redundantly and writes the full output.
"""

import numpy as np

import concourse.bacc as bacc
import concourse.bass as bass
import concourse.mybir as mybir
import concourse.tile as tile
from concourse.bass_utils import run_bass_kernel_spmd
from concourse.masks import make_identity

T, B, C, N, CR = 4, 16, 512, 1024, 64
NCORES = 8
BL = B // NCORES            # batch rows per core (2)
CB = C // 128               # 128-partition channel blocks (4)
ROWS = T * BL               # local (t, b) rows (8)
GR = NCORES * BL            # gathered rows per timestep (16)
TBALL = T * B               # total batch rows for BN (64)
BN_EPS = 1e-5

F32 = mybir.dt.float32
BF16 = mybir.dt.bfloat16
OP = mybir.AluOpType
AF = mybir.ActivationFunctionType


def _emit(tc, ctx, repeat=1, tail_repeat=1, single=False, tail_stage=99,
          use_cc=True, use_k1=True):
    nc = tc.nc
    x = nc.dram_tensor("x", [T, BL, C, N], F32, kind="ExternalInput").ap()
    w1t = nc.dram_tensor("w1t", [C, CR], F32, kind="ExternalInput").ap()
    w2t = nc.dram_tensor("w2t", [CR, C], F32, kind="ExternalInput").ap()
    g1 = nc.dram_tensor("gamma1", [CR, 1], F32, kind="ExternalInput").ap()
    g2 = nc.dram_tensor("gamma2", [1, C], F32, kind="ExternalInput").ap()
    be2 = nc.dram_tensor("beta2", [1, C], F32, kind="ExternalInput").ap()
    out = nc.dram_tensor("out", [T, B, C], F32, kind="ExternalOutput").ap()

    consts = ctx.enter_context(tc.tile_pool(name="consts", bufs=1))
    xpool = ctx.enter_context(tc.tile_pool(name="xp", bufs=5))
    ypool = ctx.enter_context(tc.tile_pool(name="yp", bufs=3))
    qpool = ctx.enter_context(tc.tile_pool(name="qp", bufs=3))
    mpool = ctx.enter_context(tc.tile_pool(name="mp", bufs=8))
    spool = ctx.enter_context(tc.tile_pool(name="sp", bufs=1))
    gpool = ctx.enter_context(tc.tile_pool(name="gp", bufs=2))
    rpool = ctx.enter_context(tc.tile_pool(name="rp", bufs=4))
    tpool = ctx.enter_context(tc.tile_pool(name="tp", bufs=2))
    psA = ctx.enter_context(tc.tile_pool(name="psA", bufs=2, space="PSUM"))
    psB = ctx.enter_context(tc.tile_pool(name="psB", bufs=1, space="PSUM"))
    psC = ctx.enter_context(tc.tile_pool(name="psC", bufs=1, space="PSUM"))
    dram = ctx.enter_context(tc.tile_pool(name="dr", bufs=1, space="DRAM"))

    # ---- constants / weights (overlaps with the streaming phase) ----
    ident = consts.tile([128, 128], F32)
    make_identity(nc, ident)
    w1t_sb = consts.tile([128, CB, CR], F32)
    for cb in range(CB):
        nc.sync.dma_start(w1t_sb[:, cb, :], w1t[cb * 128:(cb + 1) * 128, :])
    w2t_sb = consts.tile([CR, C], F32)
    nc.sync.dma_start(w2t_sb[:], w2t[:])
    g1_sb = consts.tile([CR, 1], F32)
    nc.sync.dma_start(g1_sb[:], g1[:])
    g2_sb = consts.tile([1, C], F32)
    nc.sync.dma_start(g2_sb[:], g2[:])
    be2_sb = consts.tile([1, C], F32)
    nc.sync.dma_start(be2_sb[:], be2[:])
    be2b = consts.tile([TBALL, C], F32)
    nc.gpsimd.partition_broadcast(be2b[:], be2_sb[:])
    ones_bf = consts.tile([TBALL, 1], BF16)
    nc.vector.memset(ones_bf[:], 1.0)
    ones_row = consts.tile([1, TBALL], F32)
    nc.vector.memset(ones_row[:], 1.0)
    eps_sb = consts.tile([128, 1], F32)
    nc.vector.memset(eps_sb[:], BN_EPS)
    warm_sb = consts.tile([128, 1], F32)
    # warm the Sqrt activation table during the streaming phase
    nc.scalar.activation(warm_sb[:], eps_sb[:], AF.Sqrt, bias=eps_sb[:])

    # persistent cc buffers, one pair per timestep so gathers overlap
    cc_in = [dram.tile([BL, CR], F32, tag=f"ci{t}", name=f"cc_in{t}")
             for t in range(T)]
    cc_out = [dram.tile([GR, CR], F32, tag=f"co{t}", name=f"cc_out{t}")
              for t in range(T)]

    # ---- streaming LIF + GAP, with the per-t conv1/gather pipelined ----
    # stats[:, cb, t, b] = sum_n (P_t < theta_t)  (count of NON-spikes)
    stats = spool.tile([128, CB, T, BL], F32)
    # h1T accumulates gathered, transposed h1 columns in (t, m, b) order
    h1T = spool.tile([CR, TBALL], F32, tag="h1T", name="h1T")
    st24 = spool.tile([CR, T, nc.vector.BN_STATS_DIM], F32, tag="st24",
                      name="st24")
    pstate = [spool.tile([128, BL, N], BF16, tag=f"P{cb}", name=f"P{cb}")
              for cb in range(CB)]
    masks = [None] * CB

    def consume(t):
        # collective t finished ≥1 timestep (~11us) ago: no queue stall.
        h1blk = rpool.tile([GR, CR], F32, tag=f"h1b{t}", name=f"h1b{t}")
        nc.scalar.dma_start(h1blk[:], cc_out[t][:])
        h1Tp = psA.tile([CR, GR], F32, tag="h1Tp", name=f"h1Tp{t}")
        nc.tensor.transpose(h1Tp[:], h1blk[:], ident[:GR, :GR])
        nc.vector.tensor_copy(h1T[:, t * GR:(t + 1) * GR], h1Tp[:])
        nc.vector.bn_stats(st24[:, t, :], h1T[:, t * GR:(t + 1) * GR])

    for _rep in range(repeat):
      for t in range(T):
          for cb in range(CB):
              P = pstate[cb]
              xt = xpool.tile([128, BL, N], F32)
              src = x[t, :, cb * 128:(cb + 1) * 128, :].rearrange("b c n -> c b n")
              nc.sync.dma_start(xt[:], src)
              if t == 0:
                  # P_1 = x_1 (cast to bf16)
                  nc.scalar.activation(P[:], xt[:], AF.Copy, scale=1.0)
              else:
                  y = ypool.tile([128, BL, N], BF16)
                  nc.scalar.activation(y[:], xt[:], AF.Copy, scale=float(2 ** t))
                  q = qpool.tile([128, BL, N], BF16)
                  nc.vector.tensor_mul(q[:], P[:], masks[cb][:])
                  nc.vector.tensor_add(P[:], q[:], y[:])
              m = mpool.tile([128, BL, N], BF16)
              theta = float(2 ** (t + 1))
              for b in range(BL):
                  nc.vector.tensor_scalar(
                      out=m[:, b, :],
                      in0=P[:, b, :],
                      scalar1=theta,
                      scalar2=None,
                      op0=OP.is_lt,
                      op1=OP.add,
                      accum_out=stats[:, cb, t, b:b + 1],
                  )
              masks[cb] = m

          # ---- timestep t fully counted: conv1 rows + gather, overlapped
          # with timestep t+1 streaming (serial only for t = T-1) ----
          gm = gpool.tile([128, CB, BL], F32, tag=f"gm{t}", name=f"gm{t}")
          nc.vector.tensor_scalar(
              out=gm[:], in0=stats[:, :, t, :], scalar1=-1.0 / N, scalar2=1.0,
              op0=OP.mult, op1=OP.add,
          )
          h1p = psA.tile([BL, CR], F32, tag="h1p", name=f"h1p{t}")
          for cb in range(CB):
              nc.tensor.matmul(h1p[:], gm[:, cb, :], w1t_sb[:, cb, :],
                               start=(cb == 0), stop=(cb == CB - 1))
          # h1row copy on DVE (brief wait on the PE matmul) so the ACT
          # queue never depends on end-of-timestep DVE work
          h1row = rpool.tile([BL, CR], F32, tag=f"h1r{t}", name=f"h1r{t}")
          nc.vector.tensor_copy(h1row[:], h1p[:])
          nc.sync.dma_start(cc_in[t][:], h1row[:])
          if single or not use_cc:
              for _slot in range(NCORES):
                  nc.sync.dma_start(
                      cc_out[t][BL * _slot:BL * (_slot + 1), :], cc_in[t][:])
          else:
              nc.gpsimd.collective_compute(
                  "AllGather", OP.bypass,
                  replica_groups=[list(range(NCORES))],
                  ins=[cc_in[t][:].opt()], outs=[cc_out[t][:].opt()],
              )
          if t >= 1:
              consume(t - 1)
      consume(T - 1)

    # ---- serial BN tail: only this remains after streaming ends ----
    for _trep in range(tail_repeat):
      if tail_stage < 1:
          gm2 = gpool.tile([128, CB, BL], F32, tag="gmx", name="gmx")
          nc.vector.tensor_scalar(
              out=gm2[:], in0=stats[:, :, T - 1, :], scalar1=-1.0 / N,
              scalar2=1.0, op0=OP.mult, op1=OP.add)
          continue
      # BN1 stats over all 64 rows; d1 = gamma1/std1, msh = mean1*d1
      mv1 = tpool.tile([CR, nc.vector.BN_AGGR_DIM], F32, tag="mv1", name="mv1")
      nc.vector.bn_aggr(mv1[:], st24[:].rearrange("p t s -> p (t s)"))
      std1 = tpool.tile([CR, 1], F32, tag="std1", name="std1")
      nc.scalar.activation(std1[:], mv1[:, 1:2], AF.Sqrt, bias=eps_sb[:CR])
      d1 = tpool.tile([CR, 1], F32, tag="d1", name="d1")
      nc.vector.reciprocal(d1[:], std1[:])
      nc.vector.tensor_mul(d1[:], d1[:], g1_sb[:])
      msh = tpool.tile([CR, 1], F32, tag="msh", name="msh")
      nc.vector.tensor_mul(msh[:], mv1[:, 0:1], d1[:])
      # h1c = (h1 - mean1) * d1   (columns already in output (t,m,b) order)
      h1cT = tpool.tile([CR, TBALL], F32, tag="h1cT", name="h1cT")
      nc.vector.tensor_scalar(
          out=h1cT[:], in0=h1T[:], scalar1=d1[:], scalar2=msh[:],
          op0=OP.mult, op1=OP.subtract,
      )
      if tail_stage < 2:
          continue
      # h2c = h1c @ w2.T is exactly BN2-centered: mean_rows(h2c) == 0
      h2c = psB.tile([TBALL, C], F32, tag="h2c", name="h2c")
      nc.tensor.matmul(h2c[:], h1cT[:], w2t_sb[:], start=True, stop=True)
      # var2 = mean_rows(h2c^2) via one ACT square (scaled 1/8) + ones-matmul
      h2sq = tpool.tile([TBALL, C], BF16, tag="h2sq", name="h2sq")
      nc.scalar.activation(h2sq[:], h2c[:], AF.Square, scale=0.125)
      if tail_stage < 3:
          continue
      q2 = psC.tile([1, C], F32, tag="q2", name="q2")
      nc.tensor.matmul(q2[:], ones_bf[:], h2sq[:], start=True, stop=True)
      std2 = tpool.tile([1, C], F32, tag="std2", name="std2")
      nc.scalar.activation(std2[:], q2[:], AF.Sqrt, bias=eps_sb[:1])
      d2 = tpool.tile([1, C], F32, tag="d2", name="d2")
      nc.vector.reciprocal_approx_fast(d2[:], std2[:])
      nc.vector.tensor_mul(d2[:], d2[:], g2_sb[:])
      if tail_stage < 4:
          continue
      # broadcast d2 across the 64 row-partitions with a K=1 matmul
      d2b = tpool.tile([TBALL, C], F32, tag="d2bs", name="d2b")
      if use_k1:
          d2b_ps = psC.tile([TBALL, C], F32, tag="d2b", name="d2b_ps")
          nc.tensor.matmul(d2b_ps[:], ones_row[:], d2[:], start=True, stop=True)
          nc.scalar.copy(d2b[:], d2b_ps[:])
      else:
          nc.gpsimd.partition_broadcast(d2b[:], d2[:])
      # out = h2c * d2 + beta2, rows already in output order
      o2 = tpool.tile([TBALL, C], F32, tag="o2", name="o2")
      nc.vector.tensor_mul(o2[:], h2c[:], d2b[:])
      outf = tpool.tile([TBALL, C], F32, tag="outf", name="outf")
      nc.vector.tensor_add(outf[:], o2[:], be2b[:])
      if tail_stage < 5:
          continue

      nc.sync.dma_start(out[:].rearrange("t b c -> (t b) c"), outf[:])


_CACHE = {}


def _build(repeat=1, tail_repeat=1, single=False, tail_stage=99,
           use_cc=True, use_k1=True):
    key = ("nc", repeat, tail_repeat, single, tail_stage, use_cc, use_k1)
    if key in _CACHE:
        return _CACHE[key]
    from contextlib import ExitStack
    nc = bacc.Bacc("TRN2", target_bir_lowering=False, debug=False,
                   num_devices=1 if single else NCORES)
    with tile.TileContext(nc) as tc, ExitStack() as ctx:
        _emit(tc, ctx, repeat=repeat, tail_repeat=tail_repeat, single=single,
              tail_stage=tail_stage, use_cc=use_cc, use_k1=use_k1)
    nc.compile()
    _CACHE[key] = nc
    return nc


def make_in_maps(x, w1, gamma1, beta1, w2, gamma2, beta2):
    # beta1 cancels exactly inside BN2's mean subtraction; unused on device.
    x = np.ascontiguousarray(np.asarray(x, dtype=np.float32))
    w1t = np.ascontiguousarray(np.asarray(w1, np.float32).T)
    w2t = np.ascontiguousarray(np.asarray(w2, np.float32).T)
    g1 = np.asarray(gamma1, np.float32).reshape(CR, 1)
    g2 = np.asarray(gamma2, np.float32).reshape(1, C)
    be2 = np.asarray(beta2, np.float32).reshape(1, C)
    return [
        {
            "x": np.ascontiguousarray(x[:, BL * m:BL * (m + 1)]),
            "w1t": w1t, "w2t": w2t,
            "gamma1": g1,
            "gamma2": g2, "beta2": be2,
        }
        for m in range(NCORES)
    ]


def kernel(x, w1, b1, gamma1, beta1, w2, b2, gamma2, beta2):
    # b1/b2/beta1 cancel exactly inside the following batch-norms; unused.
    nc = _build()
    in_maps = make_in_maps(x, w1, gamma1, beta1, w2, gamma2, beta2)
    res = run_bass_kernel_spmd(nc, in_maps, core_ids=list(range(NCORES)))
    out = res.results[0]["out"]
    return np.asarray(out, np.float32).reshape(T, B, C, 1)


# revision 13
# speedup vs baseline: 1.1106x; 1.1106x over previous
"""Trainium2 Bass kernel for ChannelAttention-SNN (LIF -> GAP -> 1x1conv -> BN
-> 1x1conv -> BN).

Contract: kernel(**inputs) takes the FULL unsharded inputs (as produced by
setup_inputs) and returns the FULL output [T, B, C, 1] float32.

Strategy (hardcoded for T=4, B=16, C=512, N=1024, Cr=64, 8 cores):
  - Data-parallel over B: core m processes b in {2m, 2m+1}.
  - LIF scan is unrolled over T in "P-space": P_t = 2^t * v_pre_t, so
      P_t = P_{t-1} * m_{t-1} + 2^{t-1} * x_t,   spike_t <=> P_t >= 2^t,
    which folds the 1/tau decay into the (free) scale of the ScalarE cast
    fp32->bf16. Per timestep the VectorE does: mask = (P < theta) with a
    fused free-dim count (accum_out) that directly yields the GAP sums,
    a mask multiply, and an add.
  - The conv/BN tail runs in fp32 on the PE/DVE: per-core h1 partial rows
    [8, 64], one AllGather, then every core redundantly computes the
    batch-norm tail for all 64 rows and writes the full output (batch-stat
    all-reduce is subsumed by the gather; outputs are identical across
    cores).
"""

import numpy as np

import concourse.bacc as bacc
import concourse.bass as bass
import concourse.mybir as mybir
import concourse.tile as tile
from concourse.bass_utils import run_bass_kernel_spmd
from concourse.masks import make_identity

T, B, C, N, CR = 4, 16, 512, 1024, 64
NCORES = 8
BL = B // NCORES            # batch rows per core (2)
CB = C // 128               # 128-partition channel blocks (4)
ROWS = T * BL               # local (t, b) rows (8)
TBALL = T * B               # total batch rows for BN (64)
BN_EPS = 1e-5

F32 = mybir.dt.float32
BF16 = mybir.dt.bfloat16
OP = mybir.AluOpType
AF = mybir.ActivationFunctionType
AX = mybir.AxisListType


def _emit(tc, ctx, repeat=1, tail_repeat=1, single=False, tail_stage=99):
    nc = tc.nc
    x = nc.dram_tensor("x", [T, BL, C, N], F32, kind="ExternalInput").ap()
    w1t = nc.dram_tensor("w1t", [C, CR], F32, kind="ExternalInput").ap()
    w2t = nc.dram_tensor("w2t", [CR, C], F32, kind="ExternalInput").ap()
    g1 = nc.dram_tensor("gamma1", [CR, 1], F32, kind="ExternalInput").ap()
    be1 = nc.dram_tensor("beta1", [CR, 1], F32, kind="ExternalInput").ap()
    g2 = nc.dram_tensor("gamma2", [1, C], F32, kind="ExternalInput").ap()
    be2 = nc.dram_tensor("beta2", [1, C], F32, kind="ExternalInput").ap()
    out = nc.dram_tensor("out", [T, B, C], F32, kind="ExternalOutput").ap()

    consts = ctx.enter_context(tc.tile_pool(name="consts", bufs=1))
    xpool = ctx.enter_context(tc.tile_pool(name="xp", bufs=5))
    ypool = ctx.enter_context(tc.tile_pool(name="yp", bufs=3))
    qpool = ctx.enter_context(tc.tile_pool(name="qp", bufs=3))
    mpool = ctx.enter_context(tc.tile_pool(name="mp", bufs=8))
    spool = ctx.enter_context(tc.tile_pool(name="sp", bufs=1))
    tpool = ctx.enter_context(tc.tile_pool(name="tp", bufs=2))
    psum = ctx.enter_context(tc.tile_pool(name="ps", bufs=1, space="PSUM"))
    psum2 = ctx.enter_context(tc.tile_pool(name="ps2", bufs=1, space="PSUM"))
    dram = ctx.enter_context(tc.tile_pool(name="dr", bufs=1, space="DRAM"))

    # ---- constants / weights (overlaps with the streaming phase) ----
    ident = consts.tile([128, 128], F32)
    make_identity(nc, ident)
    w1t_sb = consts.tile([128, CB, CR], F32)
    for cb in range(CB):
        nc.sync.dma_start(w1t_sb[:, cb, :], w1t[cb * 128:(cb + 1) * 128, :])
    w2t_sb = consts.tile([CR, C], F32)
    nc.sync.dma_start(w2t_sb[:], w2t[:])
    g1_sb = consts.tile([CR, 1], F32)
    nc.sync.dma_start(g1_sb[:], g1[:])
    be1_sb = consts.tile([CR, 1], F32)
    nc.sync.dma_start(be1_sb[:], be1[:])
    g2_sb = consts.tile([1, C], F32)
    nc.sync.dma_start(g2_sb[:], g2[:])
    be2_sb = consts.tile([1, C], F32)
    nc.sync.dma_start(be2_sb[:], be2[:])
    ones_row = consts.tile([1, TBALL], F32)
    nc.vector.memset(ones_row[:], 1.0)
    be2b = consts.tile([TBALL, C], F32)
    nc.gpsimd.partition_broadcast(be2b[:], be2_sb[:])
    ones_sb = consts.tile([TBALL, 1], F32)
    nc.vector.memset(ones_sb[:], 1.0)
    eps_sb = consts.tile([128, 1], F32)
    nc.vector.memset(eps_sb[:], BN_EPS)
    warm_sb = consts.tile([128, 1], F32)
    # warm the Sqrt activation table during the streaming phase
    nc.scalar.activation(warm_sb[:], eps_sb[:], AF.Sqrt, bias=eps_sb[:])
    ones_bf = consts.tile([TBALL, 1], BF16)
    nc.vector.memset(ones_bf[:], 1.0)

    # ---- streaming LIF + GAP ----
    # stats[:, cb, t, b] = sum_n (P_t < theta_t)  (count of NON-spikes)
    stats = spool.tile([128, CB, T, BL], F32)
    pstate = [spool.tile([128, BL, N], BF16, tag=f"P{cb}", name=f"P{cb}")
              for cb in range(CB)]
    masks = [None] * CB

    for _rep in range(repeat):
      for t in range(T):
          for cb in range(CB):
              P = pstate[cb]
              xt = xpool.tile([128, BL, N], F32)
              src = x[t, :, cb * 128:(cb + 1) * 128, :].rearrange("b c n -> c b n")
              nc.sync.dma_start(xt[:], src)
              if t == 0:
                  # P_1 = x_1 (cast to bf16)
                  nc.scalar.activation(P[:], xt[:], AF.Copy, scale=1.0)
              else:
                  y = ypool.tile([128, BL, N], BF16)
                  nc.scalar.activation(y[:], xt[:], AF.Copy, scale=float(2 ** t))
                  q = qpool.tile([128, BL, N], BF16)
                  nc.vector.tensor_mul(q[:], P[:], masks[cb][:])
                  nc.vector.tensor_add(P[:], q[:], y[:])
              m = mpool.tile([128, BL, N], BF16)
              theta = float(2 ** (t + 1))
              for b in range(BL):
                  nc.vector.tensor_scalar(
                      out=m[:, b, :],
                      in0=P[:, b, :],
                      scalar1=theta,
                      scalar2=None,
                      op0=OP.is_lt,
                      op1=OP.add,
                      accum_out=stats[:, cb, t, b:b + 1],
                  )
              masks[cb] = m

    # ---- g = 1 - stats/N ; h1 partial rows = g @ w1.T  (per-core rows) ----
    for _trep in range(tail_repeat):
      gm = spool.tile([128, CB, T, BL], F32, tag="gm", name="gm")
      nc.vector.tensor_scalar(
          out=gm[:], in0=stats[:], scalar1=-1.0 / N, scalar2=1.0,
          op0=OP.mult, op1=OP.add,
      )
      if tail_stage < 1:
          continue
      h1_ps = psum.tile([ROWS, CR], F32, tag="h1")
      for cb in range(CB):
          nc.tensor.matmul(
              h1_ps[:],
              gm[:, cb].rearrange("p t b -> p (t b)"),
              w1t_sb[:, cb, :],
              start=(cb == 0),
              stop=(cb == CB - 1),
          )
      h1_sb = tpool.tile([ROWS, CR], F32, tag="h1s")
      nc.vector.tensor_copy(h1_sb[:], h1_ps[:])

      # ---- AllGather local h1 rows -> all 64 batch rows on every core ----
      if tail_stage < 2:
          continue
      cc_in = dram.tile([ROWS, CR], F32)
      cc_out = dram.tile([TBALL, CR], F32)
      nc.sync.dma_start(cc_in[:], h1_sb[:])
      if single:
          for _slot in range(NCORES):
              nc.sync.dma_start(cc_out[ROWS * _slot:ROWS * (_slot + 1), :],
                                cc_in[:])
      else:
          nc.gpsimd.collective_compute(
              "AllGather", OP.bypass,
              replica_groups=[list(range(NCORES))],
              ins=[cc_in[:].opt()], outs=[cc_out[:].opt()],
          )
      h1_all = tpool.tile([TBALL, CR], F32, tag="h1a")
      nc.sync.dma_start(h1_all[:], cc_out[:])

      if tail_stage < 3:
          continue
      # ---- lean BN tail: beta1/b1/b2 cancel; h2c = h1c @ w2.T is
      # already BN2-centered, so no mu2 machinery is needed ----
      h1T_ps = psum.tile([CR, TBALL], F32, tag="tr", name="h1T_ps")
      nc.tensor.transpose(h1T_ps[:], h1_all[:], ident[:TBALL, :TBALL])
      h1T = tpool.tile([CR, TBALL], F32, tag="h1T", name="h1T")
      nc.vector.tensor_copy(h1T[:], h1T_ps[:])

      st6 = tpool.tile([CR, nc.vector.BN_STATS_DIM], F32, tag="st6", name="st6")
      nc.vector.bn_stats(st6[:], h1T[:])
      mv1 = tpool.tile([CR, nc.vector.BN_AGGR_DIM], F32, tag="mv1", name="mv1")
      nc.vector.bn_aggr(mv1[:], st6[:])
      std1 = tpool.tile([CR, 1], F32, tag="std1", name="std1")
      nc.scalar.activation(std1[:], mv1[:, 1:2], AF.Sqrt, bias=eps_sb[:CR])
      d1 = tpool.tile([CR, 1], F32, tag="d1", name="d1")
      nc.vector.reciprocal(d1[:], std1[:])
      nc.vector.tensor_mul(d1[:], d1[:], g1_sb[:])
      msh = tpool.tile([CR, 1], F32, tag="msh", name="msh")
      nc.vector.tensor_mul(msh[:], mv1[:, 0:1], d1[:])
      # h1c columns permuted from gathered (m, t, b) into output (t, m, b)
      h1cT = tpool.tile([CR, TBALL], F32, tag="h1cT", name="h1cT")
      h1cT_wr = h1cT[:].rearrange("j (t m b) -> j m t b", t=T, m=NCORES, b=BL)
      nc.vector.tensor_scalar(
          out=h1cT_wr, in0=h1T[:], scalar1=d1[:], scalar2=msh[:],
          op0=OP.mult, op1=OP.subtract,
      )
      if tail_stage < 4:
          continue
      h2c = psum.tile([TBALL, C], F32, tag="h2", name="h2c")
      nc.tensor.matmul(h2c[:], h1cT[:], w2t_sb[:], start=True, stop=True)
      h2sq = tpool.tile([TBALL, C], BF16, tag="h2sq", name="h2sq")
      nc.scalar.activation(h2sq[:], h2c[:], AF.Square, scale=0.125)
      if tail_stage < 5:
          continue
      q2 = psum2.tile([1, C], F32, tag="q2", name="q2")
      nc.tensor.matmul(q2[:], ones_bf[:], h2sq[:], start=True, stop=True)
      std2 = tpool.tile([1, C], F32, tag="std2", name="std2")
      nc.scalar.activation(std2[:], q2[:], AF.Sqrt, bias=eps_sb[:1])
      d2 = tpool.tile([1, C], F32, tag="d2", name="d2")
      nc.vector.reciprocal_approx_fast(d2[:], std2[:])
      nc.vector.tensor_mul(d2[:], d2[:], g2_sb[:])
      if tail_stage < 6:
          continue
      d2b_ps = psum2.tile([TBALL, C], F32, tag="d2b", name="d2b_ps")
      nc.tensor.matmul(d2b_ps[:], ones_row[:], d2[:], start=True, stop=True)
      d2b = tpool.tile([TBALL, C], F32, tag="d2bs", name="d2b")
      nc.scalar.copy(d2b[:], d2b_ps[:])
      o2 = tpool.tile([TBALL, C], F32, tag="o2", name="o2")
      nc.vector.tensor_mul(o2[:], h2c[:], d2b[:])
      outf = tpool.tile([TBALL, C], F32, tag="outf", name="outf")
      nc.vector.tensor_add(outf[:], o2[:], be2b[:])
      if tail_stage < 7:
          continue

      nc.sync.dma_start(out[:].rearrange("t b c -> (t b) c"), outf[:])


_CACHE = {}


def _build(repeat=1, tail_repeat=1, single=False, tail_stage=99):
    key = ("nc", repeat, tail_repeat, single, tail_stage)
    if key in _CACHE:
        return _CACHE[key]
    from contextlib import ExitStack
    nc = bacc.Bacc("TRN2", target_bir_lowering=False, debug=False,
                   num_devices=1 if single else NCORES)
    with tile.TileContext(nc) as tc, ExitStack() as ctx:
        _emit(tc, ctx, repeat=repeat, tail_repeat=tail_repeat, single=single, tail_stage=tail_stage)
    nc.compile()
    _CACHE[key] = nc
    return nc


def make_in_maps(x, w1, gamma1, beta1, w2, gamma2, beta2):
    x = np.ascontiguousarray(np.asarray(x, dtype=np.float32))
    w1t = np.ascontiguousarray(np.asarray(w1, np.float32).T)
    w2t = np.ascontiguousarray(np.asarray(w2, np.float32).T)
    g1 = np.asarray(gamma1, np.float32).reshape(CR, 1)
    be1 = np.asarray(beta1, np.float32).reshape(CR, 1)
    g2 = np.asarray(gamma2, np.float32).reshape(1, C)
    be2 = np.asarray(beta2, np.float32).reshape(1, C)
    return [
        {
            "x": np.ascontiguousarray(x[:, BL * m:BL * (m + 1)]),
            "w1t": w1t, "w2t": w2t,
            "gamma1": g1, "beta1": be1,
            "gamma2": g2, "beta2": be2,
        }
        for m in range(NCORES)
    ]


def kernel(x, w1, b1, gamma1, beta1, w2, b2, gamma2, beta2):
    # b1/b2 cancel exactly inside the following batch-norms; unused.
    nc = _build()
    in_maps = make_in_maps(x, w1, gamma1, beta1, w2, gamma2, beta2)
    res = run_bass_kernel_spmd(nc, in_maps, core_ids=list(range(NCORES)))
    out = res.results[0]["out"]
    return np.asarray(out, np.float32).reshape(T, B, C, 1)



# revision 14
# speedup vs baseline: 1.1532x; 1.0384x over previous
"""Trainium2 Bass kernel for ChannelAttention-SNN (LIF -> GAP -> 1x1conv -> BN
-> 1x1conv -> BN).

Contract: kernel(**inputs) takes the FULL unsharded inputs (as produced by
setup_inputs) and returns the FULL output [T, B, C, 1] float32.

Strategy (hardcoded for T=4, B=16, C=512, N=1024, Cr=64, 8 cores):
  - Data-parallel over B: core m processes b in {2m, 2m+1}.
  - LIF scan is unrolled over T in "P-space": P_t = 2^t * v_pre_t, so
      P_t = P_{t-1} * m_{t-1} + 2^{t-1} * x_t,   spike_t <=> P_t >= 2^t,
    which folds the 1/tau decay into the (free) scale of the ScalarE cast
    fp32->bf16. Per timestep the VectorE does: mask = (P < theta) with a
    fused free-dim count (accum_out) that directly yields the GAP sums,
    a mask multiply, and an add.
  - The conv/BN tail runs in fp32 on the PE/DVE: per-core h1 partial rows
    [8, 64], one AllGather, then every core redundantly computes the
    batch-norm tail for all 64 rows and writes the full output (batch-stat
    all-reduce is subsumed by the gather; outputs are identical across
    cores).
"""

import numpy as np

import concourse.bacc as bacc
import concourse.bass as bass
import concourse.mybir as mybir
import concourse.tile as tile
from concourse.bass_utils import run_bass_kernel_spmd
from concourse.masks import make_identity

T, B, C, N, CR = 4, 16, 512, 1024, 64
NCORES = 8
BL = B // NCORES            # batch rows per core (2)
CB = C // 128               # 128-partition channel blocks (4)
ROWS = T * BL               # local (t, b) rows (8)
TBALL = T * B               # total batch rows for BN (64)
BN_EPS = 1e-5

F32 = mybir.dt.float32
BF16 = mybir.dt.bfloat16
OP = mybir.AluOpType
AF = mybir.ActivationFunctionType
AX = mybir.AxisListType


def _emit(tc, ctx, repeat=1, tail_repeat=1, single=False, tail_stage=99):
    nc = tc.nc
    x = nc.dram_tensor("x", [T, BL, C, N], F32, kind="ExternalInput").ap()
    w1t = nc.dram_tensor("w1t", [C, CR], F32, kind="ExternalInput").ap()
    w2t = nc.dram_tensor("w2t", [CR, C], F32, kind="ExternalInput").ap()
    g1 = nc.dram_tensor("gamma1", [CR, 1], F32, kind="ExternalInput").ap()
    be1 = nc.dram_tensor("beta1", [CR, 1], F32, kind="ExternalInput").ap()
    g2 = nc.dram_tensor("gamma2", [1, C], F32, kind="ExternalInput").ap()
    be2 = nc.dram_tensor("beta2", [1, C], F32, kind="ExternalInput").ap()
    out = nc.dram_tensor("out", [T, B, C], F32, kind="ExternalOutput").ap()

    consts = ctx.enter_context(tc.tile_pool(name="consts", bufs=1))
    xpool = ctx.enter_context(tc.tile_pool(name="xp", bufs=5))
    ypool = ctx.enter_context(tc.tile_pool(name="yp", bufs=3))
    qpool = ctx.enter_context(tc.tile_pool(name="qp", bufs=3))
    mpool = ctx.enter_context(tc.tile_pool(name="mp", bufs=8))
    spool = ctx.enter_context(tc.tile_pool(name="sp", bufs=1))
    tpool = ctx.enter_context(tc.tile_pool(name="tp", bufs=2))
    psum = ctx.enter_context(tc.tile_pool(name="ps", bufs=1, space="PSUM"))
    psum2 = ctx.enter_context(tc.tile_pool(name="ps2", bufs=1, space="PSUM"))
    dram = ctx.enter_context(tc.tile_pool(name="dr", bufs=1, space="DRAM"))

    # ---- constants / weights (overlaps with the streaming phase) ----
    ident = consts.tile([128, 128], F32)
    make_identity(nc, ident)
    w1t_sb = consts.tile([128, CB, CR], F32)
    for cb in range(CB):
        nc.sync.dma_start(w1t_sb[:, cb, :], w1t[cb * 128:(cb + 1) * 128, :])
    w2t_sb = consts.tile([CR, C], F32)
    nc.sync.dma_start(w2t_sb[:], w2t[:])
    g1_sb = consts.tile([CR, 1], F32)
    nc.sync.dma_start(g1_sb[:], g1[:])
    be1_sb = consts.tile([CR, 1], F32)
    nc.sync.dma_start(be1_sb[:], be1[:])
    g2_sb = consts.tile([1, C], F32)
    nc.sync.dma_start(g2_sb[:], g2[:])
    be2_sb = consts.tile([1, C], F32)
    nc.sync.dma_start(be2_sb[:], be2[:])
    # BN1 guarantees mean(h1n) == beta1, so BN2's channel mean is known
    # ahead of time: mu2 = beta1 @ w2.T (+b2, which cancels).
    mu2_ps = psum2.tile([1, C], F32, tag="mu2p", name="mu2_ps")
    nc.tensor.matmul(mu2_ps[:], be1_sb[:], w2t_sb[:], start=True, stop=True)
    mu2row = consts.tile([1, C], F32)
    nc.vector.tensor_scalar_mul(mu2row[:], mu2_ps[:], 1.0 / 1.0)
    mu2sq = consts.tile([1, C], F32)
    nc.vector.tensor_mul(mu2sq[:], mu2row[:], mu2row[:])
    mu2b = consts.tile([TBALL, C], F32)
    nc.gpsimd.partition_broadcast(mu2b[:], mu2row[:])
    be2b = consts.tile([TBALL, C], F32)
    nc.gpsimd.partition_broadcast(be2b[:], be2_sb[:])
    ones_sb = consts.tile([TBALL, 1], F32)
    nc.vector.memset(ones_sb[:], 1.0)
    eps_sb = consts.tile([128, 1], F32)
    nc.vector.memset(eps_sb[:], BN_EPS)
    warm_sb = consts.tile([128, 1], F32)
    # warm the Sqrt activation table during the streaming phase
    nc.scalar.activation(warm_sb[:], eps_sb[:], AF.Sqrt, bias=eps_sb[:])
    ones_bf = consts.tile([TBALL, 1], BF16)
    nc.vector.memset(ones_bf[:], 1.0)

    # ---- streaming LIF + GAP ----
    # stats[:, cb, t, b] = sum_n (P_t < theta_t)  (count of NON-spikes)
    stats = spool.tile([128, CB, T, BL], F32)
    pstate = [spool.tile([128, BL, N], BF16, tag=f"P{cb}", name=f"P{cb}")
              for cb in range(CB)]
    masks = [None] * CB

    for _rep in range(repeat):
      for t in range(T):
          for cb in range(CB):
              P = pstate[cb]
              xt = xpool.tile([128, BL, N], F32)
              src = x[t, :, cb * 128:(cb + 1) * 128, :].rearrange("b c n -> c b n")
              nc.sync.dma_start(xt[:], src)
              if t == 0:
                  # P_1 = x_1 (cast to bf16)
                  nc.scalar.activation(P[:], xt[:], AF.Copy, scale=1.0)
              else:
                  y = ypool.tile([128, BL, N], BF16)
                  nc.scalar.activation(y[:], xt[:], AF.Copy, scale=float(2 ** t))
                  q = qpool.tile([128, BL, N], BF16)
                  nc.vector.tensor_mul(q[:], P[:], masks[cb][:])
                  nc.vector.tensor_add(P[:], q[:], y[:])
              m = mpool.tile([128, BL, N], BF16)
              theta = float(2 ** (t + 1))
              for b in range(BL):
                  nc.vector.tensor_scalar(
                      out=m[:, b, :],
                      in0=P[:, b, :],
                      scalar1=theta,
                      scalar2=None,
                      op0=OP.is_lt,
                      op1=OP.add,
                      accum_out=stats[:, cb, t, b:b + 1],
                  )
              masks[cb] = m

    # ---- g = 1 - stats/N ; h1 partial rows = g @ w1.T  (per-core rows) ----
    for _trep in range(tail_repeat):
      gm = spool.tile([128, CB, T, BL], F32, tag="gm", name="gm")
      nc.vector.tensor_scalar(
          out=gm[:], in0=stats[:], scalar1=-1.0 / N, scalar2=1.0,
          op0=OP.mult, op1=OP.add,
      )
      if tail_stage < 1:
          continue
      h1_ps = psum.tile([ROWS, CR], F32, tag="h1")
      for cb in range(CB):
          nc.tensor.matmul(
              h1_ps[:],
              gm[:, cb].rearrange("p t b -> p (t b)"),
              w1t_sb[:, cb, :],
              start=(cb == 0),
              stop=(cb == CB - 1),
          )
      h1_sb = tpool.tile([ROWS, CR], F32, tag="h1s")
      nc.vector.tensor_copy(h1_sb[:], h1_ps[:])

      # ---- AllGather local h1 rows -> all 64 batch rows on every core ----
      if tail_stage < 2:
          continue
      cc_in = dram.tile([ROWS, CR], F32)
      cc_out = dram.tile([TBALL, CR], F32)
      nc.sync.dma_start(cc_in[:], h1_sb[:])
      if single:
          for _slot in range(NCORES):
              nc.sync.dma_start(cc_out[ROWS * _slot:ROWS * (_slot + 1), :],
                                cc_in[:])
      else:
          nc.gpsimd.collective_compute(
              "AllGather", OP.bypass,
              replica_groups=[list(range(NCORES))],
              ins=[cc_in[:].opt()], outs=[cc_out[:].opt()],
          )
      h1_all = tpool.tile([TBALL, CR], F32, tag="h1a")
      nc.sync.dma_start(h1_all[:], cc_out[:])

      if tail_stage < 3:
          continue
      # ---- BN1 (stats over the 64 batch rows), in [j, tb] layout ----
      h1T_ps = psum.tile([CR, TBALL], F32, tag="tr", name="h1T_ps")
      nc.tensor.transpose(h1T_ps[:], h1_all[:], ident[:TBALL, :TBALL])
      h1T = tpool.tile([CR, TBALL], F32, tag="h1T", name="h1T")
      nc.vector.tensor_copy(h1T[:], h1T_ps[:])

      st6 = tpool.tile([CR, nc.vector.BN_STATS_DIM], F32, tag="st6", name="st6")
      nc.vector.bn_stats(st6[:], h1T[:])
      mv1 = tpool.tile([CR, nc.vector.BN_AGGR_DIM], F32, tag="mv1", name="mv1")
      nc.vector.bn_aggr(mv1[:], st6[:])
      std1 = tpool.tile([CR, 1], F32, tag="std1", name="std1")
      nc.scalar.activation(std1[:], mv1[:, 1:2], AF.Sqrt, bias=eps_sb[:CR])
      d1 = tpool.tile([CR, 1], F32, tag="d1", name="d1")
      nc.vector.reciprocal(d1[:], std1[:])
      nc.vector.tensor_mul(d1[:], d1[:], g1_sb[:])
      sh1 = tpool.tile([CR, 1], F32, tag="sh1", name="sh1")
      nc.vector.tensor_mul(sh1[:], mv1[:, 0:1], d1[:])
      nc.vector.tensor_sub(sh1[:], be1_sb[:], sh1[:])
      # write h1n columns permuted from gathered order (m, t, b) into
      # output order (t, m, b) so stage-B produces rows ready for one DMA
      h1nT = tpool.tile([CR, TBALL], F32, tag="h1nT", name="h1nT")
      h1nT_wr = h1nT[:].rearrange("j (t m b) -> j m t b", t=T, m=NCORES, b=BL)
      nc.vector.tensor_scalar(
          out=h1nT_wr, in0=h1T[:], scalar1=d1[:], scalar2=sh1[:],
          op0=OP.mult, op1=OP.add,
      )

      if tail_stage < 4:
          continue
      # ---- h2 = h1n @ w2.T  -> [64 rows, 512 ch] ----
      h2_ps = psum.tile([TBALL, C], F32, tag="h2", name="h2_ps")
      nc.tensor.matmul(h2_ps[:], h1nT[:], w2t_sb[:], start=True, stop=True)
      h2 = tpool.tile([TBALL, C], F32, tag="h2s", name="h2")
      nc.vector.tensor_copy(h2[:], h2_ps[:])
      # scale by 1/8 so sum(h2b^2) over the 64 rows is directly E[h2^2]
      h2b = tpool.tile([TBALL, C], BF16, tag="h2b", name="h2b")
      nc.scalar.activation(h2b[:], h2_ps[:], AF.Copy, scale=0.125)
      h2sq = tpool.tile([TBALL, C], BF16, tag="h2sq", name="h2sq")
      nc.vector.tensor_mul(h2sq[:], h2b[:], h2b[:])
      if tail_stage < 5:
          continue

      # ---- BN2: var over channels via one bf16 ones-matmul ----
      q2_ps = psum2.tile([1, C], F32, tag="q2", name="q2_ps")
      nc.tensor.matmul(q2_ps[:], ones_bf[:], h2sq[:], start=True, stop=True)
      var2 = tpool.tile([1, C], F32, tag="var2", name="var2")
      nc.vector.tensor_sub(var2[:], q2_ps[:], mu2sq[:])
      std2 = tpool.tile([1, C], F32, tag="std2", name="std2")
      nc.scalar.activation(std2[:], var2[:], AF.Sqrt, bias=eps_sb[:1])
      d2 = tpool.tile([1, C], F32, tag="d2", name="d2")
      nc.vector.reciprocal_approx_fast(d2[:], std2[:])
      nc.vector.tensor_mul(d2[:], d2[:], g2_sb[:])
      if tail_stage < 6:
          continue
      d2b = tpool.tile([TBALL, C], F32, tag="d2b", name="d2b")
      nc.gpsimd.partition_broadcast(d2b[:], d2[:])

      # out = (h2 - mu2) * d2 + beta2, rows already in output order
      o1 = tpool.tile([TBALL, C], F32, tag="o1", name="o1")
      nc.vector.tensor_sub(o1[:], h2[:], mu2b[:])
      o2 = tpool.tile([TBALL, C], F32, tag="o2", name="o2")
      nc.vector.tensor_mul(o2[:], o1[:], d2b[:])
      outf = tpool.tile([TBALL, C], F32, tag="outf", name="outf")
      nc.vector.tensor_add(outf[:], o2[:], be2b[:])
      if tail_stage < 7:
          continue

      nc.sync.dma_start(out[:].rearrange("t b c -> (t b) c"), outf[:])


_CACHE = {}


def _build(repeat=1, tail_repeat=1, single=False, tail_stage=99):
    key = ("nc", repeat, tail_repeat, single, tail_stage)
    if key in _CACHE:
        return _CACHE[key]
    from contextlib import ExitStack
    nc = bacc.Bacc("TRN2", target_bir_lowering=False, debug=False,
                   num_devices=1 if single else NCORES)
    with tile.TileContext(nc) as tc, ExitStack() as ctx:
        _emit(tc, ctx, repeat=repeat, tail_repeat=tail_repeat, single=single, tail_stage=tail_stage)
    nc.compile()
    _CACHE[key] = nc
    return nc


def make_in_maps(x, w1, gamma1, beta1, w2, gamma2, beta2):
    x = np.ascontiguousarray(np.asarray(x, dtype=np.float32))
    w1t = np.ascontiguousarray(np.asarray(w1, np.float32).T)
    w2t = np.ascontiguousarray(np.asarray(w2, np.float32).T)
    g1 = np.asarray(gamma1, np.float32).reshape(CR, 1)
    be1 = np.asarray(beta1, np.float32).reshape(CR, 1)
    g2 = np.asarray(gamma2, np.float32).reshape(1, C)
    be2 = np.asarray(beta2, np.float32).reshape(1, C)
    return [
        {
            "x": np.ascontiguousarray(x[:, BL * m:BL * (m + 1)]),
            "w1t": w1t, "w2t": w2t,
            "gamma1": g1, "beta1": be1,
            "gamma2": g2, "beta2": be2,
        }
        for m in range(NCORES)
    ]


def kernel(x, w1, b1, gamma1, beta1, w2, b2, gamma2, beta2):
    # b1/b2 cancel exactly inside the following batch-norms; unused.
    nc = _build()
    in_maps = make_in_maps(x, w1, gamma1, beta1, w2, gamma2, beta2)
    res = run_bass_kernel_spmd(nc, in_maps, core_ids=list(range(NCORES)))
    out = res.results[0]["out"]
    return np.asarray(out, np.float32).reshape(T, B, C, 1)

